# revision 99
# baseline (speedup 1.0000x reference)
"""Multi-head attention (B=2, S=2048, D=1024, H=16) on 8 trn2 NeuronCores.

Sharding: core c handles batch b = c//4 and heads 4*(c%4) .. 4*(c%4)+4
(tensor-parallel over heads, data-parallel over batch). Each core computes
its 4 heads' contribution to the output projection; the host sums the 4
partials per batch element and adds wo_b.

All device matmuls run in bf16 (1 PE cycle/row vs 4 for fp32):
  - host pre-transposes and casts q,k,v -> qT/kT/vT bf16 (D, S), weights
    -> bf16; the binary mask ships as uint8 maskT (Sk, Sq) -- half the
    fill-phase DMA bytes -- and is expanded to bf16 on the otherwise-idle
    GpSimd engine. Output is bf16, with the 4 per-core partials summed
    (plus wo_b) on the host in f32.
  - q/k projections produce qh/kh (128 = 2 heads x 64, S) with the bias
    folded into the matmul as a rank-1 (bias x ones) accumulation step.
  - v projection produces vp (S-chunk, 4 heads x [64 v-cols + ones-col]);
    the ones column yields the softmax denominator for free during PV.
  - scores are computed transposed per head: alphaT (Sk-chunk 128, Sq 1024)
    = k-chunk^T q, exp on ScalarE (PSUM -> SBUF bf16, the critical engine:
    128 x ~1.04us), binary-mask multiply on VectorE.
  - PV runs in the [sq, hd] orientation (scores chunk as stationary, v as
    moving): out (128 sq, 65) accumulated over 16 Sk chunks in PSUM. This
    halves PE rows vs the [hd, sq] orientation (full 128-partition fill).
    PV trails the exp front by LAG iterations so the PE in-order queue
    never waits on the exp->mask chain.
  - matmul start=True zeroes the WHOLE psum bank here, so multi-slot banks
    (PV, transposes) are zeroed once via a thin full-bank-WAR matmul and
    accumulated with start=False.
  - normalize: batched reciprocal of the denominator columns + per-
    partition tensor_scalar multiply (both DVE; GpSimd cannot read PSUM)
    -> x_sb (sq, hd) bf16.
  - x is flipped to (hd, sq) with PE transposes batched per head pair into
    one psum bank, then the output projection contracts both head pairs in
    one PSUM accumulation; half 0 overlaps half 1's attention via deferred
    "extras", half 1 drains at the end across four psum copy chains.
Emission order is everything: compute is interleaved into the attention
sk-loop via scheduled extras, DMs are emitted just-in-time (the tile
scheduler conservatively gates compute on every earlier-emitted DMA), and
~110 tiny warmup matmuls keep the PE p-state ramped through the fill.
"""

import numpy as np

B, S, D, H = 2, 2048, 1024, 16
DH = D // H  # 64
HEADS_PER_CORE = 4
N_CORES = 8
KC = 8  # D chunks of 128
NSK = 16  # Sk chunks of 128
NHALF = 2  # Sq halves of 1024 (attention granularity)

_NC = None  # cached compiled bass program


def _build():
    import concourse.mybir as mybir
    import concourse.tile as tile
    from concourse import bacc

    F32 = mybir.dt.float32
    BF16 = mybir.dt.bfloat16
    U8 = mybir.dt.uint8
    P = 128

    nc = bacc.Bacc("TRN2")

    qT = nc.dram_tensor("qT", [D, S], BF16, kind="ExternalInput")
    kT = nc.dram_tensor("kT", [D, S], BF16, kind="ExternalInput")
    vT = nc.dram_tensor("vT", [D, S], BF16, kind="ExternalInput")
    maskT = nc.dram_tensor("maskT", [S, S], U8, kind="ExternalInput")
    wqT = nc.dram_tensor("wqT", [D, 256], BF16, kind="ExternalInput")
    wkT = nc.dram_tensor("wkT", [D, 256], BF16, kind="ExternalInput")
    wvT = nc.dram_tensor("wvT", [D, 256], BF16, kind="ExternalInput")
    woT = nc.dram_tensor("woT", [256, D], BF16, kind="ExternalInput")
    wqb = nc.dram_tensor("wqb", [256], BF16, kind="ExternalInput")
    wkb = nc.dram_tensor("wkb", [256], BF16, kind="ExternalInput")
    wvb = nc.dram_tensor("wvb", [256], BF16, kind="ExternalInput")
    out = nc.dram_tensor("out", [S, D], BF16, kind="ExternalOutput")

    AF = mybir.ActivationFunctionType
    MUL = mybir.AluOpType.mult

    with tile.TileContext(nc) as tc:
        with (
            tc.tile_pool(name="persist", bufs=1) as persist,
            tc.tile_pool(name="xs", bufs=5) as xs,
            tc.tile_pool(name="mu", bufs=5) as mu,
            tc.tile_pool(name="psbp", bufs=9) as psbp,
            tc.tile_pool(name="xbuf", bufs=2) as xbuf,
            tc.tile_pool(name="osbp", bufs=9) as osbp,
            tc.tile_pool(name="rbuf", bufs=6) as rbuf,
            tc.tile_pool(name="ps_proj", bufs=2, space="PSUM") as ps_proj,
            tc.tile_pool(name="ps_alpha", bufs=2, space="PSUM") as ps_alpha,
            tc.tile_pool(name="ps_pv", bufs=2, space="PSUM") as ps_pv,
        ):
            # ---------------- persistent SBUF tiles ----------------
            wqT_sb = persist.tile([P, KC, 256], BF16, tag="wqT")
            wkT_sb = persist.tile([P, KC, 256], BF16, tag="wkT")
            wvT_sb = persist.tile([P, KC, 256], BF16, tag="wvT")
            woT_sb = persist.tile([P, 2, D], BF16, tag="woT")
            wqb_sb = persist.tile([1, 256], BF16, tag="wqb")
            wkb_sb = persist.tile([1, 256], BF16, tag="wkb")
            wvb_sb = persist.tile([1, 256], BF16, tag="wvb")
            ones_sb = persist.tile([1, 512], BF16, tag="ones")
            qh_sb = [
                persist.tile([P, S], BF16, tag=f"qh{p}", name=f"qh{p}")
                for p in range(2)
            ]
            kh_sb = [
                persist.tile([P, S], BF16, tag=f"kh{p}", name=f"kh{p}")
                for p in range(2)
            ]
            vp_sb = [
                persist.tile([P, 4, 65], BF16, tag=f"vp{sk}", name=f"vp{sk}")
                for sk in range(NSK)
            ]
            mask_sb = [
                persist.tile([P, S], BF16, tag=f"mask{sk}", name=f"mask{sk}")
                for sk in range(NSK)
            ]

            ident_sb = persist.tile([P, P], BF16, tag="ident")
            zeros_sb = persist.tile([1, P], BF16, tag="zeros")
            from concourse import masks as _masks

            _masks.make_identity(nc, ident_sb[:])
            nc.gpsimd.memset(ones_sb[:], 1.0)
            nc.gpsimd.memset(zeros_sb[:], 0.0)
            for sk in range(NSK):
                # ones column (col 64 per head); cols 0:64 are overwritten
                nc.gpsimd.memset(vp_sb[sk][:], 1.0)

            def load_stream(src, sb, nm):
                """one [128, KC, 512] tile for s-block sb (single DMA)."""
                t = xs.tile([P, KC, 512], BF16, tag="xs", name=f"{nm}{sb}")
                nc.sync.dma_start(
                    t[:],
                    src[:, 512 * sb : 512 * (sb + 1)].rearrange(
                        "(kc p) s -> p kc s", p=P
                    ),
                )
                return t

            # streams are DMA'd just-in-time (see dma_sched below): the tile
            # scheduler's batched waits gate compute on every DMA emitted
            # before it in program order, so a big upfront DMA block stalls
            # the pipeline on transfers it doesn't need yet.
            qstream = {}
            kstream = {}
            vstream = {}

            def dma_qs(sb):
                qstream[sb] = load_stream(qT, sb, "q")

            def dma_ks(sb):
                kstream[sb] = load_stream(kT, sb, "k")

            def dma_vs(sb):
                vstream[sb] = load_stream(vT, sb, "v")

            def dma_mask(m, half):
                # the mask is binary: ship it as uint8 (half the fill-phase
                # DMA bytes) and expand to bf16 on the otherwise-idle GpSimd
                mt = mu.tile([P, 1024], U8, tag="mu", name=f"mu{m}_{half}")
                nc.sync.dma_start(
                    mt[:],
                    maskT[P * m : P * (m + 1), 1024 * half : 1024 * (half + 1)],
                )
                nc.gpsimd.tensor_copy(
                    out=mask_sb[m][:, 1024 * half : 1024 * (half + 1)], in_=mt[:]
                )

            def dma_w(wsb, w):
                nc.sync.dma_start(wsb[:], w[:].rearrange("(kc p) m -> p kc m", p=P))

            def dma_wo():
                nc.sync.dma_start(
                    woT_sb[:], woT[:].rearrange("(pr p) m -> p pr m", p=P)
                )

            # ---------------- projection emitters ----------------
            def emit_qk_proj_part(which, sb, p, copy_eng=None):
                """q/k projection for s-block sb, head-pair p."""
                wsb, bsb, dst, src = {
                    "q": (wqT_sb, wqb_sb, qh_sb, qstream),
                    "k": (wkT_sb, wkb_sb, kh_sb, kstream),
                }[which]
                pps = ps_proj.tile(
                    [P, 512], F32, tag="proj", name=f"{which}ps{sb}_{p}"
                )
                for kc in range(KC):
                    nc.tensor.matmul(
                        pps[:],
                        wsb[:, kc, P * p : P * (p + 1)],
                        src[sb][:, kc, :],
                        start=(kc == 0),
                        stop=False,
                    )
                # bias via rank-1 accumulation: out += bias x ones
                nc.tensor.matmul(
                    pps[:],
                    bsb[0:1, P * p : P * (p + 1)],
                    ones_sb[0:1, :],
                    start=False,
                    stop=True,
                )
                # upfront groups copy on ACT (idle pre-attention); the
                # mid-stream groups copy on DVE to keep ACT exp-only
                if copy_eng == "dve":
                    nc.any.tensor_copy(
                        out=dst[p][:, 512 * sb : 512 * (sb + 1)], in_=pps[:]
                    )
                else:
                    nc.scalar.copy(dst[p][:, 512 * sb : 512 * (sb + 1)], pps[:])

            def emit_qk_proj(which, sb, copy_eng=None):
                for p in range(2):
                    emit_qk_proj_part(which, sb, p, copy_eng)

            def emit_qk_quarter(which, sb, p, cq):
                """one 128-col quarter of a q/k projection part: sized to the
                per-iteration PE slack so deferred parts never stall QK."""
                wsb, bsb, dst, srcs = {
                    "q": (wqT_sb, wqb_sb, qh_sb, qstream),
                    "k": (wkT_sb, wkb_sb, kh_sb, kstream),
                }[which]
                pps = ps_proj.tile(
                    [P, P], F32, tag="proj", name=f"{which}q{sb}_{p}_{cq}"
                )
                for kc in range(KC):
                    nc.tensor.matmul(
                        pps[:],
                        wsb[:, kc, P * p : P * (p + 1)],
                        srcs[sb][:, kc, P * cq : P * (cq + 1)],
                        start=(kc == 0),
                        stop=False,
                    )
                nc.tensor.matmul(
                    pps[:],
                    bsb[0:1, P * p : P * (p + 1)],
                    ones_sb[0:1, 0:P],
                    start=False,
                    stop=True,
                )
                nc.vector.tensor_copy(
                    out=dst[p][:, 512 * sb + P * cq : 512 * sb + P * (cq + 1)],
                    in_=pps[:],
                )

            def emit_v_proj(sc):
                """v projection for s-chunk sc (128 rows) -> vp_sb[sc]."""
                vps = ps_proj.tile([P, 512], F32, tag="proj", name=f"vps{sc}")
                for kc in range(KC):
                    nc.tensor.matmul(
                        vps[:, 0:256],
                        vstream[sc // 4][:, kc, P * (sc % 4) : P * (sc % 4 + 1)],
                        wvT_sb[:, kc, :],
                        start=(kc == 0),
                        stop=False,
                    )
                nc.tensor.matmul(
                    vps[:, 0:256],
                    ones_sb[0:1, 0:P],
                    wvb_sb[0:1, :],
                    start=False,
                    stop=True,
                )
                # GpSimd cannot read PSUM; let the scheduler place the copy
                nc.any.tensor_copy(
                    out=vp_sb[sc][:, :, 0:64],
                    in_=vps[:, 0:256].rearrange("p (h d) -> p h d", h=4),
                )

            # deferred PE work, injected one group per sk iteration
            extras = []

            def emit_extras(n=1):
                for _ in range(n):
                    if extras:
                        extras.pop(0)()

            # ---------------- PE warmup ----------------
            # The cost model runs the PE at 0.65/1.2 GHz until it has been
            # continuously busy for 3us. Tiny spin matmuls during the initial
            # DMA fill keep the array ramped so the projections run at 2.4GHz.
            warm_ps = ps_proj.tile([1, 64], F32, tag="proj", name="warm_ps")
            for w in range(110):
                nc.tensor.matmul(
                    warm_ps[:],
                    ones_sb[0:1, 0:1],
                    ones_sb[0:1, 0:64],
                    start=True,
                    stop=True,
                )

            # ---------------- upfront projections (DMA just ahead) ---------
            # only what the first QK needs; the rest interleaves into the
            # attention sk-loop as extras (one group per iteration, ordered so
            # every producer is emitted before its first PE consumer)
            dma_w(wqT_sb, wqT)
            nc.sync.dma_start(wqb_sb[:], wqb[:][None, :])
            dma_qs(0)
            emit_qk_proj("q", 0)
            dma_w(wkT_sb, wkT)
            nc.sync.dma_start(wkb_sb[:], wkb[:][None, :])
            dma_ks(0)
            emit_qk_proj("k", 0)
            dma_qs(1)
            emit_qk_proj("q", 1)
            dma_w(wvT_sb, wvT)
            nc.sync.dma_start(wvb_sb[:], wvb[:][None, :])
            dma_vs(0)
            dma_mask(0, 0)
            dma_mask(1, 0)

            # scheduled extras: global iteration -> deferred PE work. v/k
            # groups are just-in-time for the first head's PV/QK; q2/q3 (only
            # needed at half 1, iter 64) run after the fill-phase DMA backlog
            # clears so their stream loads never stall the PE queue.
            # one sub-1us group per iteration; k/q pair-1 parts are not
            # consumed until head h2 (iter 32) / half-1 h2 (iter 96), so they
            # run in the post-fill slack instead of bursting the fill
            sched = [
                ("v", 0), ("v", 1), ("k", 1, 0), ("v", 2), ("v", 3),
                ("v", 4), ("k", 2, 0), ("v", 5), ("v", 6), ("v", 7),
                ("k", 3, 0), ("v", 8), ("v", 9), ("v", 10), ("v", 11),
                ("v", 12), ("v", 13), ("v", 14), ("v", 15),
            ] + [("kq", sb, 1, cq) for sb in (1, 2, 3) for cq in range(4)]
            extra_sched = {}
            for i, item in enumerate(sched):
                if item[0] == "v":
                    fn = lambda sc=item[1]: emit_v_proj(sc)
                elif item[0] == "kq":
                    fn = lambda sb=item[1], p=item[2], c=item[3]: (
                        emit_qk_quarter("k", sb, p, c)
                    )
                else:
                    fn = lambda sb=item[1], p=item[2]: emit_qk_proj_part(
                        "k", sb, p, "dve"
                    )
                extra_sched.setdefault(i, []).append(fn)
            extra_sched[40] = [lambda: emit_qk_proj_part("q", 2, 0, "dve")]
            extra_sched[44] = [lambda: emit_qk_proj_part("q", 3, 0, "dve")]
            for i, cq in enumerate(range(4)):
                extra_sched[48 + i] = [
                    lambda c=cq: emit_qk_quarter("q", 2, 1, c)
                ]
                extra_sched[56 + i] = [
                    lambda c=cq: emit_qk_quarter("q", 3, 1, c)
                ]

            # just-in-time DMA schedule: global iteration -> emissions.
            # Producers must precede consumers in each queue, but emitting a
            # DMA also (conservatively) gates later-emitted compute, so each
            # transfer lands only a few iterations before first use.
            dma_sched = {
                0: [lambda: dma_mask(2, 0), lambda: dma_mask(3, 0)],
                1: [lambda: dma_ks(1)],
                2: [lambda: dma_vs(1), lambda: dma_mask(4, 0)],
                3: [lambda: dma_mask(5, 0), lambda: dma_mask(6, 0)],
                4: [lambda: dma_mask(7, 0)],
                5: [lambda: dma_ks(2)],
                6: [lambda: dma_vs(2), lambda: dma_mask(8, 0)],
                7: [lambda: dma_mask(9, 0), lambda: dma_mask(10, 0)],
                8: [lambda: dma_mask(11, 0)],
                9: [lambda: dma_ks(3)],
                10: [lambda: dma_mask(12, 0), lambda: dma_mask(13, 0)],
                11: [lambda: dma_mask(14, 0), lambda: dma_mask(15, 0)],
                12: [lambda: dma_vs(3)],
                13: [lambda: dma_qs(2)],
                14: [lambda: dma_qs(3)],
                31: [lambda: dma_wo()],
            }
            for j in range(16):
                dma_sched.setdefault(15 + j, []).append(
                    lambda m=j: dma_mask(m, 1)
                )

            # ---------------- attention + output projection ----------------
            def emit_pv(pv, psb, sk, hl):
                """PV matmuls for score chunk sk: 8 sq-tiles of 128.

                start=True zeroes the WHOLE psum bank in this executor, so a
                bank with 4 packed accumulation slots gets one explicit
                zeroing matmul; the slot accumulations all run start=False.
                """
                if sk == 0:
                    for g in range(2):
                        # start=True zeroes the whole bank irrespective of the
                        # out width; a 1-col-per-slot out keeps the cost at 4
                        # rows while registering WAR against every slot reader
                        nc.tensor.matmul(
                            pv[g][:, :, 0:1],
                            zeros_sb[:],
                            ones_sb[0:1, 0:4],
                            start=True,
                            stop=False,
                            skip_group_check=True,
                        )
                for t in range(8):
                    nc.tensor.matmul(
                        pv[t // 4][:, t % 4, 0:65],
                        psb[:, P * t : P * (t + 1)],
                        vp_sb[sk][:, hl, :],
                        start=False,
                        stop=(sk == NSK - 1),
                        skip_group_check=True,
                    )

            def emit_oproj_db(half, t, db, osb):
                """one 512-col piece of the output projection for sq-tile t."""
                tt = 8 * half + t
                ops = ps_proj.tile(
                    [P, 512], F32, tag="proj", name=f"ops{tt}_{db}"
                )
                for pr in range(2):
                    nc.tensor.matmul(
                        ops[:],
                        xT_sb[half][pr][:, t, :],
                        woT_sb[:, pr, 512 * db : 512 * (db + 1)],
                        start=(pr == 0),
                        stop=(pr == 1),
                    )
                nc.any.tensor_copy(
                    out=osb[:, 512 * db : 512 * (db + 1)], in_=ops[:]
                )
                # fine-grained out DMA so the tail drains per 512-col piece
                nc.sync.dma_start(
                    out[P * tt : P * (tt + 1), 512 * db : 512 * (db + 1)],
                    osb[:, 512 * db : 512 * (db + 1)],
                )

            def emit_oproj_drain(t, osb):
                """half-1 O-proj at the drain: all engines are otherwise idle.
                Even tiles use 2-bank alpha-slot psum + one ACT copy; odd
                tiles use two proj-slot pieces + DVE copies. Four independent
                psum chains keep the drain PE-bound."""
                tt = 8 + t
                if t % 2 == 0:
                    ops = ps_alpha.tile(
                        [P, D], F32, tag="alpha", name=f"opsd{t}"
                    )
                    for db in range(2):
                        for pr in range(2):
                            nc.tensor.matmul(
                                ops[:, 512 * db : 512 * (db + 1)],
                                xT_sb[1][pr][:, t, :],
                                woT_sb[:, pr, 512 * db : 512 * (db + 1)],
                                start=(pr == 0),
                                stop=(pr == 1),
                            )
                    nc.scalar.copy(osb[:], ops[:])
                else:
                    for db in range(2):
                        ops = ps_proj.tile(
                            [P, 512], F32, tag="proj", name=f"opsd{t}_{db}"
                        )
                        for pr in range(2):
                            nc.tensor.matmul(
                                ops[:],
                                xT_sb[1][pr][:, t, :],
                                woT_sb[:, pr, 512 * db : 512 * (db + 1)],
                                start=(pr == 0),
                                stop=(pr == 1),
                            )
                        nc.vector.tensor_copy(
                            out=osb[:, 512 * db : 512 * (db + 1)], in_=ops[:]
                        )
                nc.sync.dma_start(out[P * tt : P * (tt + 1), :], osb[:])

            # Flattened attention over (half, head, sk): PV lags one iteration
            # globally (also across head boundaries) so the PE never sits
            # behind the exp->mask chain of the current sk; normalize and the
            # half-end work are emitted inside the next iterations.
            xT_sb = {}  # half -> [pr] tiles
            x_sb = {}  # half -> tile
            for half in range(NHALF):
                x_sb[half] = xbuf.tile([P, 8, 256], BF16, tag="x", name=f"x{half}")

            def emit_normalize(half, hl, pv):
                """r = 1/denom (one batched DVE recip per pv tile), then
                x = pv * r per sq-tile (DVE: GpSimd cannot read PSUM)."""
                for g in range(2):
                    r = rbuf.tile(
                        [P, 4, 1], F32, tag="r", name=f"r{half}_{hl}_{g}"
                    )
                    nc.vector.reciprocal(r[:], pv[g][:, :, 64:65])
                    for i in range(4):
                        t = 4 * g + i
                        nc.vector.tensor_scalar(
                            x_sb[half][:, t, 64 * hl : 64 * hl + 64],
                            pv[g][:, i, 0:64],
                            r[:, i, :],
                            None,
                            MUL,
                        )

            def emit_transposes(half, p):
                """x (sq, hd) -> xT (hd, sq) for head pair p: 8 PE transposes
                into one PSUM tile (shares the "pv" slot rotation), one DVE
                copy out."""
                if half not in xT_sb:
                    xT_sb[half] = [
                        xbuf.tile(
                            [P, 8, P], BF16, tag=f"xT{q}", name=f"xT{half}_{q}"
                        )
                        for q in range(2)
                    ]
                tp = ps_proj.tile([P, 512], F32, tag="proj", name=f"tp{half}_{p}")
                nc.tensor.matmul(
                    tp[:].rearrange("p (t s) -> p t s", t=8)[:, :, 0:1],
                    zeros_sb[:],
                    ones_sb[0:1, 0:8],
                    start=True,
                    stop=False,
                    skip_group_check=True,
                )
                tpb = tp[:].bitcast(BF16)
                for t in range(8):
                    nc.tensor.matmul(
                        tpb[:, P * t : P * (t + 1)],
                        x_sb[half][:, t, P * p : P * (p + 1)],
                        ident_sb[:],
                        is_transpose=True,
                        start=False,
                        stop=(t == 7),
                        skip_group_check=True,
                    )
                nc.vector.tensor_copy(
                    out=xT_sb[half][p][:].rearrange("p t s -> p (t s)"), in_=tpb
                )

            iters = [
                (half, hl, sk)
                for half in range(NHALF)
                for hl in range(4)
                for sk in range(NSK)
            ]
            LAG = 7  # PV trails the QK/exp front by this many iterations

            def retire(p):
                """emit deferred PV (+ head/half epilogue when sk==15)."""
                ppv, ppsb, psk, phl, phalf = p
                emit_pv(ppv, ppsb, psk, phl)
                if psk == NSK - 1:
                    emit_normalize(phalf, phl, ppv)
                    # transposes + O-proj scheduling go through post_extras so
                    # they pop after the normalize has drained on DVE
                    if phl == 1:
                        post_extras.append(
                            lambda phalf=phalf: emit_transposes(phalf, 0)
                        )
                    if phl == 3:
                        post_extras.append(
                            lambda phalf=phalf: emit_transposes(phalf, 1)
                        )
                        if phalf == 0:
                            def sched_half0():
                                for t in range(8):
                                    osb = osbp.tile(
                                        [P, D], BF16, tag="osb", name=f"osb0_{t}"
                                    )
                                    for db in range(2):
                                        extras.append(
                                            lambda t=t, osb=osb, db=db:
                                            emit_oproj_db(0, t, db, osb)
                                        )
                            post_extras.append(sched_half0)
                        else:
                            def sched_drain():
                                for t in range(8):
                                    osb = osbp.tile(
                                        [P, D], BF16, tag="osb", name=f"osbd{t}"
                                    )
                                    emit_oproj_drain(t, osb)
                            post_extras.append(sched_drain)

            pending = []
            post_extras = []
            pv_cur = None
            for it_idx, (half, hl, sk) in enumerate(iters):
                pr, hs = hl // 2, hl % 2
                for fn in extra_sched.get(it_idx, ()):
                    fn()
                emit_extras()
                # retire BEFORE this iteration's QK/exp/mask: the normalize
                # then sits ahead of the not-yet-ready mask in the DVE queue
                if len(pending) > LAG - 1 and pending:
                    retire(pending.pop(0))
                # drain the lag early near the end so the epilogue is short
                if it_idx >= 121 and pending:
                    retire(pending.pop(0))
                if sk == 0:
                    pv_cur = [
                        ps_pv.tile(
                            [P, 4, P], F32, tag="pv", name=f"pv{half}_{hl}_{g}"
                        )
                        for g in range(2)
                    ]
                alpha = ps_alpha.tile(
                    [P, 1024], F32, tag="alpha", name=f"al{half}_{hl}_{sk}"
                )
                for j in range(2):
                    nc.tensor.matmul(
                        alpha[:, 512 * j : 512 * (j + 1)],
                        kh_sb[pr][64 * hs : 64 * hs + 64, P * sk : P * (sk + 1)],
                        qh_sb[pr][
                            64 * hs : 64 * hs + 64,
                            1024 * half + 512 * j : 1024 * half + 512 * (j + 1),
                        ],
                        start=True,
                        stop=True,
                    )
                psb = psbp.tile(
                    [P, 1024], BF16, tag="psb", name=f"psb{half}_{hl}_{sk}"
                )
                nc.scalar.activation(psb[:], alpha[:], AF.Exp)
                nc.vector.tensor_tensor(
                    psb[:],
                    psb[:],
                    mask_sb[sk][:, 1024 * half : 1024 * (half + 1)],
                    MUL,
                )
                pending.append((pv_cur, psb, sk, hl, half))
                if post_extras:
                    post_extras.pop(0)()
                for fn in dma_sched.get(it_idx, ()):
                    fn()
            while pending:
                retire(pending.pop(0))
            while post_extras:
                post_extras.pop(0)()
            emit_extras(len(extras))

    nc.finalize()
    return nc


def _get_nc():
    global _NC
    if _NC is None:
        _NC = _build()
    return _NC


def _prep_inputs(q, k, v, mask, wq_w, wq_b, wk_w, wk_b, wv_w, wv_b, wo_w, wo_b):
    import ml_dtypes

    bf16 = ml_dtypes.bfloat16
    f32 = np.float32
    q = np.asarray(q, f32)
    k = np.asarray(k, f32)
    v = np.asarray(v, f32)
    mask = np.asarray(mask)
    wq_w = np.asarray(wq_w, f32)
    wk_w = np.asarray(wk_w, f32)
    wv_w = np.asarray(wv_w, f32)
    wo_w = np.asarray(wo_w, f32)

    qTb = [np.ascontiguousarray(q[b].T).astype(bf16) for b in range(B)]
    kTb = [np.ascontiguousarray(k[b].T).astype(bf16) for b in range(B)]
    vTb = [np.ascontiguousarray(v[b].T).astype(bf16) for b in range(B)]
    maskTb = [
        np.ascontiguousarray((~mask[b, 0]).T).astype(np.uint8) for b in range(B)
    ]

    in_maps = []
    for c in range(N_CORES):
        b = c // 4
        g = c % 4
        rows = slice(256 * g, 256 * (g + 1))
        in_maps.append(
            {
                "qT": qTb[b],
                "kT": kTb[b],
                "vT": vTb[b],
                "maskT": maskTb[b],
                "wqT": np.ascontiguousarray(wq_w[rows, :].T).astype(bf16),
                "wkT": np.ascontiguousarray(wk_w[rows, :].T).astype(bf16),
                "wvT": np.ascontiguousarray(wv_w[rows, :].T).astype(bf16),
                "woT": np.ascontiguousarray(wo_w[:, rows].T).astype(bf16),
                "wqb": np.ascontiguousarray(np.asarray(wq_b, f32)[rows]).astype(bf16),
                "wkb": np.ascontiguousarray(np.asarray(wk_b, f32)[rows]).astype(bf16),
                "wvb": np.ascontiguousarray(np.asarray(wv_b, f32)[rows]).astype(bf16),
            }
        )
    return in_maps


def run(inputs, trace=False):
    """Run the kernel; returns (output, BassKernelResults)."""
    from concourse.bass_utils import run_bass_kernel_spmd

    in_maps = _prep_inputs(**inputs)
    nc = _get_nc()
    res = None
    last_exc = None
    for attempt in range(3):
        try:
            res = run_bass_kernel_spmd(
                nc, in_maps, core_ids=list(range(N_CORES)), trace=trace
            )
            break
        except Exception as e:  # transient device/tunnel failures
            last_exc = e
            try:
                import jax

                jax.clear_caches()
                try:
                    jax.extend.backend.clear_backends()
                except Exception:
                    from jax._src import api as _jax_api

                    _jax_api.clear_backends()
            except Exception:
                pass
            import time as _time

            _time.sleep(2.0 * (attempt + 1))
    if res is None:
        raise last_exc
    wo_b = np.asarray(inputs["wo_b"], np.float32)
    out = np.zeros((B, S, D), np.float32)
    for b in range(B):
        acc = np.zeros((S, D), np.float32)
        for g in range(4):
            acc += np.asarray(res.results[4 * b + g]["out"], np.float32)
        out[b] = acc + wo_b[None, :]
    return out, res


def kernel(**inputs) -> np.ndarray:
    out, _ = run(inputs, trace=False)
    return out


# revision 100
# speedup vs baseline: 1.0022x; 1.0022x over previous
"""Multi-head attention (B=2, S=2048, D=1024, H=16) on 8 trn2 NeuronCores.

Sharding: core c handles batch b = c//4 and heads 4*(c%4) .. 4*(c%4)+4
(tensor-parallel over heads, data-parallel over batch). Each core computes
its 4 heads' contribution to the output projection; the host sums the 4
partials per batch element and adds wo_b.

All device matmuls run in bf16 (1 PE cycle/row vs 4 for fp32):
  - host pre-transposes and casts q,k,v -> qT/kT/vT bf16 (D, S), weights
    -> bf16; the binary mask ships as uint8 maskT (Sk, Sq) -- half the
    fill-phase DMA bytes -- and is expanded to bf16 on the otherwise-idle
    GpSimd engine. Output is bf16, with the 4 per-core partials summed
    (plus wo_b) on the host in f32.
  - q/k projections produce qh/kh (128 = 2 heads x 64, S) with the bias
    folded into the matmul as a rank-1 (bias x ones) accumulation step.
  - v projection produces vp (S-chunk, 4 heads x [64 v-cols + ones-col]);
    the ones column yields the softmax denominator for free during PV.
  - scores are computed transposed per head: alphaT (Sk-chunk 128, Sq 1024)
    = k-chunk^T q, exp on ScalarE (PSUM -> SBUF bf16, the critical engine:
    128 x ~1.04us), binary-mask multiply on VectorE.
  - PV runs in the [sq, hd] orientation (scores chunk as stationary, v as
    moving): out (128 sq, 65) accumulated over 16 Sk chunks in PSUM. This
    halves PE rows vs the [hd, sq] orientation (full 128-partition fill).
    PV trails the exp front by LAG iterations so the PE in-order queue
    never waits on the exp->mask chain.
  - matmul start=True zeroes the WHOLE psum bank here, so multi-slot banks
    (PV, transposes) are zeroed once via a thin full-bank-WAR matmul and
    accumulated with start=False.
  - normalize: batched reciprocal of the denominator columns + per-
    partition tensor_scalar multiply (both DVE; GpSimd cannot read PSUM)
    -> x_sb (sq, hd) bf16.
  - x is flipped to (hd, sq) with PE transposes batched per head pair into
    one psum bank, then the output projection contracts both head pairs in
    one PSUM accumulation; half 0 overlaps half 1's attention via deferred
    "extras", half 1 drains at the end across four psum copy chains.
Emission order is everything: compute is interleaved into the attention
sk-loop via scheduled extras, DMs are emitted just-in-time (the tile
scheduler conservatively gates compute on every earlier-emitted DMA), and
~110 tiny warmup matmuls keep the PE p-state ramped through the fill.
"""

import numpy as np

B, S, D, H = 2, 2048, 1024, 16
DH = D // H  # 64
HEADS_PER_CORE = 4
N_CORES = 8
KC = 8  # D chunks of 128
NSK = 16  # Sk chunks of 128
NHALF = 2  # Sq halves of 1024 (attention granularity)

_NC = None  # cached compiled bass program


def _build():
    import concourse.mybir as mybir
    import concourse.tile as tile
    from concourse import bacc

    F32 = mybir.dt.float32
    BF16 = mybir.dt.bfloat16
    U8 = mybir.dt.uint8
    P = 128

    nc = bacc.Bacc("TRN2")

    qT = nc.dram_tensor("qT", [D, S], BF16, kind="ExternalInput")
    kT = nc.dram_tensor("kT", [D, S], BF16, kind="ExternalInput")
    vT = nc.dram_tensor("vT", [D, S], BF16, kind="ExternalInput")
    maskT = nc.dram_tensor("maskT", [S, S], U8, kind="ExternalInput")
    wqT = nc.dram_tensor("wqT", [D, 256], BF16, kind="ExternalInput")
    wkT = nc.dram_tensor("wkT", [D, 256], BF16, kind="ExternalInput")
    wvT = nc.dram_tensor("wvT", [D, 256], BF16, kind="ExternalInput")
    woT = nc.dram_tensor("woT", [256, D], BF16, kind="ExternalInput")
    wqb = nc.dram_tensor("wqb", [256], BF16, kind="ExternalInput")
    wkb = nc.dram_tensor("wkb", [256], BF16, kind="ExternalInput")
    wvb = nc.dram_tensor("wvb", [256], BF16, kind="ExternalInput")
    out = nc.dram_tensor("out", [S, D], BF16, kind="ExternalOutput")

    AF = mybir.ActivationFunctionType
    MUL = mybir.AluOpType.mult

    with tile.TileContext(nc) as tc:
        with (
            tc.tile_pool(name="persist", bufs=1) as persist,
            tc.tile_pool(name="xs", bufs=5) as xs,
            tc.tile_pool(name="mu", bufs=5) as mu,
            tc.tile_pool(name="psbp", bufs=9) as psbp,
            tc.tile_pool(name="xbuf", bufs=2) as xbuf,
            tc.tile_pool(name="osbp", bufs=9) as osbp,
            tc.tile_pool(name="rbuf", bufs=6) as rbuf,
            tc.tile_pool(name="ps_proj", bufs=2, space="PSUM") as ps_proj,
            tc.tile_pool(name="ps_alpha", bufs=2, space="PSUM") as ps_alpha,
            tc.tile_pool(name="ps_pv", bufs=2, space="PSUM") as ps_pv,
        ):
            # ---------------- persistent SBUF tiles ----------------
            wqT_sb = persist.tile([P, KC, 256], BF16, tag="wqT")
            wkT_sb = persist.tile([P, KC, 256], BF16, tag="wkT")
            wvT_sb = persist.tile([P, KC, 256], BF16, tag="wvT")
            woT_sb = persist.tile([P, 2, D], BF16, tag="woT")
            wqb_sb = persist.tile([1, 256], BF16, tag="wqb")
            wkb_sb = persist.tile([1, 256], BF16, tag="wkb")
            wvb_sb = persist.tile([1, 256], BF16, tag="wvb")
            ones_sb = persist.tile([1, 512], BF16, tag="ones")
            qh_sb = [
                persist.tile([P, S], BF16, tag=f"qh{p}", name=f"qh{p}")
                for p in range(2)
            ]
            kh_sb = [
                persist.tile([P, S], BF16, tag=f"kh{p}", name=f"kh{p}")
                for p in range(2)
            ]
            vp_sb = [
                persist.tile([P, 4, 65], BF16, tag=f"vp{sk}", name=f"vp{sk}")
                for sk in range(NSK)
            ]
            mask_sb = [
                persist.tile([P, S], BF16, tag=f"mask{sk}", name=f"mask{sk}")
                for sk in range(NSK)
            ]

            ident_sb = persist.tile([P, P], BF16, tag="ident")
            zeros_sb = persist.tile([1, P], BF16, tag="zeros")
            from concourse import masks as _masks

            _masks.make_identity(nc, ident_sb[:])
            nc.gpsimd.memset(ones_sb[:], 1.0)
            nc.gpsimd.memset(zeros_sb[:], 0.0)
            for sk in range(NSK):
                # ones column (col 64 per head); cols 0:64 are overwritten
                nc.gpsimd.memset(vp_sb[sk][:], 1.0)

            def load_stream(src, sb, nm):
                """one [128, KC, 512] tile for s-block sb (single DMA)."""
                t = xs.tile([P, KC, 512], BF16, tag="xs", name=f"{nm}{sb}")
                nc.sync.dma_start(
                    t[:],
                    src[:, 512 * sb : 512 * (sb + 1)].rearrange(
                        "(kc p) s -> p kc s", p=P
                    ),
                )
                return t

            # streams are DMA'd just-in-time (see dma_sched below): the tile
            # scheduler's batched waits gate compute on every DMA emitted
            # before it in program order, so a big upfront DMA block stalls
            # the pipeline on transfers it doesn't need yet.
            qstream = {}
            kstream = {}
            vstream = {}

            def dma_qs(sb):
                qstream[sb] = load_stream(qT, sb, "q")

            def dma_ks(sb):
                kstream[sb] = load_stream(kT, sb, "k")

            def dma_vs(sb):
                vstream[sb] = load_stream(vT, sb, "v")

            def dma_mask(m, half):
                # the mask is binary: ship it as uint8 (half the fill-phase
                # DMA bytes) and expand to bf16 on the otherwise-idle GpSimd
                mt = mu.tile([P, 1024], U8, tag="mu", name=f"mu{m}_{half}")
                nc.sync.dma_start(
                    mt[:],
                    maskT[P * m : P * (m + 1), 1024 * half : 1024 * (half + 1)],
                )
                nc.gpsimd.tensor_copy(
                    out=mask_sb[m][:, 1024 * half : 1024 * (half + 1)], in_=mt[:]
                )

            def dma_w(wsb, w):
                nc.sync.dma_start(wsb[:], w[:].rearrange("(kc p) m -> p kc m", p=P))

            def dma_wo():
                nc.sync.dma_start(
                    woT_sb[:], woT[:].rearrange("(pr p) m -> p pr m", p=P)
                )

            # ---------------- projection emitters ----------------
            def emit_qk_proj_part(which, sb, p, copy_eng=None):
                """q/k projection for s-block sb, head-pair p."""
                wsb, bsb, dst, src = {
                    "q": (wqT_sb, wqb_sb, qh_sb, qstream),
                    "k": (wkT_sb, wkb_sb, kh_sb, kstream),
                }[which]
                pps = ps_proj.tile(
                    [P, 512], F32, tag="proj", name=f"{which}ps{sb}_{p}"
                )
                for kc in range(KC):
                    nc.tensor.matmul(
                        pps[:],
                        wsb[:, kc, P * p : P * (p + 1)],
                        src[sb][:, kc, :],
                        start=(kc == 0),
                        stop=False,
                    )
                # bias via rank-1 accumulation: out += bias x ones
                nc.tensor.matmul(
                    pps[:],
                    bsb[0:1, P * p : P * (p + 1)],
                    ones_sb[0:1, :],
                    start=False,
                    stop=True,
                )
                # upfront groups copy on ACT (idle pre-attention); the
                # mid-stream groups copy on DVE to keep ACT exp-only
                if copy_eng == "dve":
                    nc.any.tensor_copy(
                        out=dst[p][:, 512 * sb : 512 * (sb + 1)], in_=pps[:]
                    )
                else:
                    nc.scalar.copy(dst[p][:, 512 * sb : 512 * (sb + 1)], pps[:])

            def emit_qk_proj(which, sb, copy_eng=None):
                for p in range(2):
                    emit_qk_proj_part(which, sb, p, copy_eng)

            def emit_qk_quarter(which, sb, p, cq):
                """one 128-col quarter of a q/k projection part: sized to the
                per-iteration PE slack so deferred parts never stall QK."""
                wsb, bsb, dst, srcs = {
                    "q": (wqT_sb, wqb_sb, qh_sb, qstream),
                    "k": (wkT_sb, wkb_sb, kh_sb, kstream),
                }[which]
                pps = ps_proj.tile(
                    [P, P], F32, tag="proj", name=f"{which}q{sb}_{p}_{cq}"
                )
                for kc in range(KC):
                    nc.tensor.matmul(
                        pps[:],
                        wsb[:, kc, P * p : P * (p + 1)],
                        srcs[sb][:, kc, P * cq : P * (cq + 1)],
                        start=(kc == 0),
                        stop=False,
                    )
                nc.tensor.matmul(
                    pps[:],
                    bsb[0:1, P * p : P * (p + 1)],
                    ones_sb[0:1, 0:P],
                    start=False,
                    stop=True,
                )
                nc.vector.tensor_copy(
                    out=dst[p][:, 512 * sb + P * cq : 512 * sb + P * (cq + 1)],
                    in_=pps[:],
                )

            def emit_v_proj(sc):
                """v projection for s-chunk sc (128 rows) -> vp_sb[sc]."""
                vps = ps_proj.tile([P, 512], F32, tag="proj", name=f"vps{sc}")
                for kc in range(KC):
                    nc.tensor.matmul(
                        vps[:, 0:256],
                        vstream[sc // 4][:, kc, P * (sc % 4) : P * (sc % 4 + 1)],
                        wvT_sb[:, kc, :],
                        start=(kc == 0),
                        stop=False,
                    )
                nc.tensor.matmul(
                    vps[:, 0:256],
                    ones_sb[0:1, 0:P],
                    wvb_sb[0:1, :],
                    start=False,
                    stop=True,
                )
                # GpSimd cannot read PSUM; let the scheduler place the copy
                nc.any.tensor_copy(
                    out=vp_sb[sc][:, :, 0:64],
                    in_=vps[:, 0:256].rearrange("p (h d) -> p h d", h=4),
                )

            # deferred PE work, injected one group per sk iteration
            extras = []

            def emit_extras(n=1):
                for _ in range(n):
                    if extras:
                        extras.pop(0)()

            # ---------------- PE warmup ----------------
            # The cost model runs the PE at 0.65/1.2 GHz until it has been
            # continuously busy for 3us. Tiny spin matmuls during the initial
            # DMA fill keep the array ramped so the projections run at 2.4GHz.
            warm_ps = ps_proj.tile([1, 64], F32, tag="proj", name="warm_ps")
            for w in range(110):
                nc.tensor.matmul(
                    warm_ps[:],
                    ones_sb[0:1, 0:1],
                    ones_sb[0:1, 0:64],
                    start=True,
                    stop=True,
                )

            # ---------------- upfront projections (DMA just ahead) ---------
            # only what the first QK needs; the rest interleaves into the
            # attention sk-loop as extras (one group per iteration, ordered so
            # every producer is emitted before its first PE consumer)
            dma_w(wqT_sb, wqT)
            nc.sync.dma_start(wqb_sb[:], wqb[:][None, :])
            dma_qs(0)
            emit_qk_proj("q", 0)
            dma_w(wkT_sb, wkT)
            nc.sync.dma_start(wkb_sb[:], wkb[:][None, :])
            dma_ks(0)
            emit_qk_proj("k", 0)
            dma_qs(1)
            emit_qk_proj("q", 1)
            dma_w(wvT_sb, wvT)
            nc.sync.dma_start(wvb_sb[:], wvb[:][None, :])
            dma_vs(0)
            dma_mask(0, 0)
            dma_mask(1, 0)

            # scheduled extras: global iteration -> deferred PE work. v/k
            # groups are just-in-time for the first head's PV/QK; q2/q3 (only
            # needed at half 1, iter 64) run after the fill-phase DMA backlog
            # clears so their stream loads never stall the PE queue.
            # one sub-1us group per iteration; k/q pair-1 parts are not
            # consumed until head h2 (iter 32) / half-1 h2 (iter 96), so they
            # run in the post-fill slack instead of bursting the fill
            sched = [
                ("v", 0), ("v", 1), ("k", 1, 0), ("v", 2), ("v", 3),
                ("v", 4), ("k", 2, 0), ("v", 5), ("v", 6), ("v", 7),
                ("k", 3, 0), ("v", 8), ("v", 9), ("v", 10), ("v", 11),
                ("v", 12), ("v", 13), ("v", 14), ("v", 15),
            ] + [("kq", sb, 1, cq) for sb in (1, 2, 3) for cq in range(4)]
            extra_sched = {}
            for i, item in enumerate(sched):
                if item[0] == "v":
                    fn = lambda sc=item[1]: emit_v_proj(sc)
                elif item[0] == "kq":
                    fn = lambda sb=item[1], p=item[2], c=item[3]: (
                        emit_qk_quarter("k", sb, p, c)
                    )
                else:
                    fn = lambda sb=item[1], p=item[2]: emit_qk_proj_part(
                        "k", sb, p, "dve"
                    )
                extra_sched.setdefault(i, []).append(fn)
            extra_sched[40] = [lambda: emit_qk_proj_part("q", 2, 0, "dve")]
            extra_sched[44] = [lambda: emit_qk_proj_part("q", 3, 0, "dve")]
            for i, cq in enumerate(range(4)):
                extra_sched[48 + i] = [
                    lambda c=cq: emit_qk_quarter("q", 2, 1, c)
                ]
                extra_sched[56 + i] = [
                    lambda c=cq: emit_qk_quarter("q", 3, 1, c)
                ]

            # just-in-time DMA schedule: global iteration -> emissions.
            # Producers must precede consumers in each queue, but emitting a
            # DMA also (conservatively) gates later-emitted compute, so each
            # transfer lands only a few iterations before first use.
            dma_sched = {
                0: [lambda: dma_mask(2, 0), lambda: dma_mask(3, 0)],
                1: [lambda: dma_ks(1)],
                2: [lambda: dma_vs(1), lambda: dma_mask(4, 0)],
                3: [lambda: dma_mask(5, 0), lambda: dma_mask(6, 0)],
                4: [lambda: dma_mask(7, 0)],
                5: [lambda: dma_ks(2)],
                6: [lambda: dma_vs(2), lambda: dma_mask(8, 0)],
                7: [lambda: dma_mask(9, 0), lambda: dma_mask(10, 0)],
                8: [lambda: dma_mask(11, 0)],
                9: [lambda: dma_ks(3)],
                10: [lambda: dma_mask(12, 0), lambda: dma_mask(13, 0)],
                11: [lambda: dma_mask(14, 0), lambda: dma_mask(15, 0)],
                12: [lambda: dma_vs(3)],
                13: [lambda: dma_qs(2)],
                14: [lambda: dma_qs(3)],
                31: [lambda: dma_wo()],
            }
            for j in range(16):
                dma_sched.setdefault(15 + j, []).append(
                    lambda m=j: dma_mask(m, 1)
                )

            # ---------------- attention + output projection ----------------
            def emit_pv(pv, psb, sk, hl):
                """PV matmuls for score chunk sk: 8 sq-tiles of 128.

                start=True zeroes the WHOLE psum bank in this executor, so a
                bank with 4 packed accumulation slots gets one explicit
                zeroing matmul; the slot accumulations all run start=False.
                """
                if sk == 0:
                    for g in range(2):
                        # start=True zeroes the whole bank irrespective of the
                        # out width; a 1-col-per-slot out keeps the cost at 4
                        # rows while registering WAR against every slot reader
                        nc.tensor.matmul(
                            pv[g][:, :, 0:1],
                            zeros_sb[:],
                            ones_sb[0:1, 0:4],
                            start=True,
                            stop=False,
                            skip_group_check=True,
                        )
                for t in range(8):
                    nc.tensor.matmul(
                        pv[t // 4][:, t % 4, 0:65],
                        psb[:, P * t : P * (t + 1)],
                        vp_sb[sk][:, hl, :],
                        start=False,
                        stop=(sk == NSK - 1),
                        skip_group_check=True,
                    )

            def emit_oproj_db(half, t, db, osb):
                """one 512-col piece of the output projection for sq-tile t."""
                tt = 8 * half + t
                ops = ps_proj.tile(
                    [P, 512], F32, tag="proj", name=f"ops{tt}_{db}"
                )
                for pr in range(2):
                    nc.tensor.matmul(
                        ops[:],
                        xT_sb[half][pr][:, t, :],
                        woT_sb[:, pr, 512 * db : 512 * (db + 1)],
                        start=(pr == 0),
                        stop=(pr == 1),
                    )
                nc.any.tensor_copy(
                    out=osb[:, 512 * db : 512 * (db + 1)], in_=ops[:]
                )
                # fine-grained out DMA so the tail drains per 512-col piece
                nc.sync.dma_start(
                    out[P * tt : P * (tt + 1), 512 * db : 512 * (db + 1)],
                    osb[:, 512 * db : 512 * (db + 1)],
                )

            def emit_oproj_drain(t, osb):
                """half-1 O-proj at the drain: all engines are otherwise idle.
                Even tiles use 2-bank alpha-slot psum + one ACT copy; odd
                tiles use two proj-slot pieces + DVE copies. Four independent
                psum chains keep the drain PE-bound."""
                tt = 8 + t
                if t % 2 == 0:
                    ops = ps_alpha.tile(
                        [P, D], F32, tag="alpha", name=f"opsd{t}"
                    )
                    for db in range(2):
                        for pr in range(2):
                            nc.tensor.matmul(
                                ops[:, 512 * db : 512 * (db + 1)],
                                xT_sb[1][pr][:, t, :],
                                woT_sb[:, pr, 512 * db : 512 * (db + 1)],
                                start=(pr == 0),
                                stop=(pr == 1),
                            )
                    nc.scalar.copy(osb[:], ops[:])
                else:
                    for db in range(2):
                        ops = ps_proj.tile(
                            [P, 512], F32, tag="proj", name=f"opsd{t}_{db}"
                        )
                        for pr in range(2):
                            nc.tensor.matmul(
                                ops[:],
                                xT_sb[1][pr][:, t, :],
                                woT_sb[:, pr, 512 * db : 512 * (db + 1)],
                                start=(pr == 0),
                                stop=(pr == 1),
                            )
                        nc.vector.tensor_copy(
                            out=osb[:, 512 * db : 512 * (db + 1)], in_=ops[:]
                        )
                nc.sync.dma_start(out[P * tt : P * (tt + 1), :], osb[:])

            # Flattened attention over (half, head, sk): PV lags one iteration
            # globally (also across head boundaries) so the PE never sits
            # behind the exp->mask chain of the current sk; normalize and the
            # half-end work are emitted inside the next iterations.
            xT_sb = {}  # half -> [pr] tiles
            x_sb = {}  # half -> tile
            for half in range(NHALF):
                x_sb[half] = xbuf.tile([P, 8, 256], BF16, tag="x", name=f"x{half}")

            def emit_normalize(half, hl, pv):
                """r = 1/denom (one batched DVE recip per pv tile), then
                x = pv * r per sq-tile (DVE: GpSimd cannot read PSUM)."""
                for g in range(2):
                    r = rbuf.tile(
                        [P, 4, 1], F32, tag="r", name=f"r{half}_{hl}_{g}"
                    )
                    nc.vector.reciprocal(r[:], pv[g][:, :, 64:65])
                    for i in range(4):
                        t = 4 * g + i
                        nc.vector.tensor_scalar(
                            x_sb[half][:, t, 64 * hl : 64 * hl + 64],
                            pv[g][:, i, 0:64],
                            r[:, i, :],
                            None,
                            MUL,
                        )

            def emit_transposes(half, p):
                """x (sq, hd) -> xT (hd, sq) for head pair p: 8 PE transposes
                into one PSUM tile (shares the "pv" slot rotation), one DVE
                copy out."""
                if half not in xT_sb:
                    xT_sb[half] = [
                        xbuf.tile(
                            [P, 8, P], BF16, tag=f"xT{q}", name=f"xT{half}_{q}"
                        )
                        for q in range(2)
                    ]
                tp = ps_proj.tile([P, 512], F32, tag="proj", name=f"tp{half}_{p}")
                nc.tensor.matmul(
                    tp[:].rearrange("p (t s) -> p t s", t=8)[:, :, 0:1],
                    zeros_sb[:],
                    ones_sb[0:1, 0:8],
                    start=True,
                    stop=False,
                    skip_group_check=True,
                )
                tpb = tp[:].bitcast(BF16)
                xTv = xT_sb[half][p][:].rearrange("p t s -> p (t s)")
                for t in range(8):
                    nc.tensor.matmul(
                        tpb[:, P * t : P * (t + 1)],
                        x_sb[half][:, t, P * p : P * (p + 1)],
                        ident_sb[:],
                        is_transpose=True,
                        start=False,
                        stop=(t == 7),
                        skip_group_check=True,
                    )
                    if t == 3:
                        # first-half copy unblocks the O-proj for tiles 0-3
                        # while tiles 4-7 are still transposing
                        nc.vector.tensor_copy(
                            out=xTv[:, 0:512], in_=tpb[:, 0:512]
                        )
                nc.vector.tensor_copy(out=xTv[:, 512:1024], in_=tpb[:, 512:1024])

            iters = [
                (half, hl, sk)
                for half in range(NHALF)
                for hl in range(4)
                for sk in range(NSK)
            ]
            LAG = 7  # PV trails the QK/exp front by this many iterations

            def retire(p):
                """emit deferred PV (+ head/half epilogue when sk==15)."""
                ppv, ppsb, psk, phl, phalf = p
                emit_pv(ppv, ppsb, psk, phl)
                if psk == NSK - 1:
                    emit_normalize(phalf, phl, ppv)
                    # transposes + O-proj scheduling go through post_extras so
                    # they pop after the normalize has drained on DVE
                    if phl == 1:
                        post_extras.append(
                            lambda phalf=phalf: emit_transposes(phalf, 0)
                        )
                    if phl == 3:
                        post_extras.append(
                            lambda phalf=phalf: emit_transposes(phalf, 1)
                        )
                        if phalf == 0:
                            def sched_half0():
                                for t in range(8):
                                    osb = osbp.tile(
                                        [P, D], BF16, tag="osb", name=f"osb0_{t}"
                                    )
                                    for db in range(2):
                                        extras.append(
                                            lambda t=t, osb=osb, db=db:
                                            emit_oproj_db(0, t, db, osb)
                                        )
                            post_extras.append(sched_half0)
                        else:
                            def sched_drain():
                                for t in range(8):
                                    osb = osbp.tile(
                                        [P, D], BF16, tag="osb", name=f"osbd{t}"
                                    )
                                    emit_oproj_drain(t, osb)
                            post_extras.append(sched_drain)

            pending = []
            post_extras = []
            pv_cur = None
            for it_idx, (half, hl, sk) in enumerate(iters):
                pr, hs = hl // 2, hl % 2
                for fn in extra_sched.get(it_idx, ()):
                    fn()
                emit_extras()
                # retire BEFORE this iteration's QK/exp/mask: the normalize
                # then sits ahead of the not-yet-ready mask in the DVE queue
                if len(pending) > LAG - 1 and pending:
                    retire(pending.pop(0))
                # drain the lag early near the end so the epilogue is short
                if it_idx >= 121 and pending:
                    retire(pending.pop(0))
                if sk == 0:
                    pv_cur = [
                        ps_pv.tile(
                            [P, 4, P], F32, tag="pv", name=f"pv{half}_{hl}_{g}"
                        )
                        for g in range(2)
                    ]
                alpha = ps_alpha.tile(
                    [P, 1024], F32, tag="alpha", name=f"al{half}_{hl}_{sk}"
                )
                for j in range(2):
                    nc.tensor.matmul(
                        alpha[:, 512 * j : 512 * (j + 1)],
                        kh_sb[pr][64 * hs : 64 * hs + 64, P * sk : P * (sk + 1)],
                        qh_sb[pr][
                            64 * hs : 64 * hs + 64,
                            1024 * half + 512 * j : 1024 * half + 512 * (j + 1),
                        ],
                        start=True,
                        stop=True,
                    )
                psb = psbp.tile(
                    [P, 1024], BF16, tag="psb", name=f"psb{half}_{hl}_{sk}"
                )
                nc.scalar.activation(psb[:], alpha[:], AF.Exp)
                nc.vector.tensor_tensor(
                    psb[:],
                    psb[:],
                    mask_sb[sk][:, 1024 * half : 1024 * (half + 1)],
                    MUL,
                )
                pending.append((pv_cur, psb, sk, hl, half))
                if post_extras:
                    post_extras.pop(0)()
                for fn in dma_sched.get(it_idx, ()):
                    fn()
            while pending:
                retire(pending.pop(0))
            while post_extras:
                post_extras.pop(0)()
            emit_extras(len(extras))

    nc.finalize()
    return nc


def _get_nc():
    global _NC
    if _NC is None:
        _NC = _build()
    return _NC


def _prep_inputs(q, k, v, mask, wq_w, wq_b, wk_w, wk_b, wv_w, wv_b, wo_w, wo_b):
    import ml_dtypes

    bf16 = ml_dtypes.bfloat16
    f32 = np.float32
    q = np.asarray(q, f32)
    k = np.asarray(k, f32)
    v = np.asarray(v, f32)
    mask = np.asarray(mask)
    wq_w = np.asarray(wq_w, f32)
    wk_w = np.asarray(wk_w, f32)
    wv_w = np.asarray(wv_w, f32)
    wo_w = np.asarray(wo_w, f32)

    qTb = [np.ascontiguousarray(q[b].T).astype(bf16) for b in range(B)]
    kTb = [np.ascontiguousarray(k[b].T).astype(bf16) for b in range(B)]
    vTb = [np.ascontiguousarray(v[b].T).astype(bf16) for b in range(B)]
    maskTb = [
        np.ascontiguousarray((~mask[b, 0]).T).astype(np.uint8) for b in range(B)
    ]

    in_maps = []
    for c in range(N_CORES):
        b = c // 4
        g = c % 4
        rows = slice(256 * g, 256 * (g + 1))
        in_maps.append(
            {
                "qT": qTb[b],
                "kT": kTb[b],
                "vT": vTb[b],
                "maskT": maskTb[b],
                "wqT": np.ascontiguousarray(wq_w[rows, :].T).astype(bf16),
                "wkT": np.ascontiguousarray(wk_w[rows, :].T).astype(bf16),
                "wvT": np.ascontiguousarray(wv_w[rows, :].T).astype(bf16),
                "woT": np.ascontiguousarray(wo_w[:, rows].T).astype(bf16),
                "wqb": np.ascontiguousarray(np.asarray(wq_b, f32)[rows]).astype(bf16),
                "wkb": np.ascontiguousarray(np.asarray(wk_b, f32)[rows]).astype(bf16),
                "wvb": np.ascontiguousarray(np.asarray(wv_b, f32)[rows]).astype(bf16),
            }
        )
    return in_maps


def run(inputs, trace=False):
    """Run the kernel; returns (output, BassKernelResults)."""
    from concourse.bass_utils import run_bass_kernel_spmd

    in_maps = _prep_inputs(**inputs)
    nc = _get_nc()
    res = None
    last_exc = None
    for attempt in range(3):
        try:
            res = run_bass_kernel_spmd(
                nc, in_maps, core_ids=list(range(N_CORES)), trace=trace
            )
            break
        except Exception as e:  # transient device/tunnel failures
            last_exc = e
            try:
                import jax

                jax.clear_caches()
                try:
                    jax.extend.backend.clear_backends()
                except Exception:
                    from jax._src import api as _jax_api

                    _jax_api.clear_backends()
            except Exception:
                pass
            import time as _time

            _time.sleep(2.0 * (attempt + 1))
    if res is None:
        raise last_exc
    wo_b = np.asarray(inputs["wo_b"], np.float32)
    out = np.zeros((B, S, D), np.float32)
    for b in range(B):
        acc = np.zeros((S, D), np.float32)
        for g in range(4):
            acc += np.asarray(res.results[4 * b + g]["out"], np.float32)
        out[b] = acc + wo_b[None, :]
    return out, res


def kernel(**inputs) -> np.ndarray:
    out, _ = run(inputs, trace=False)
    return out
